# revision 11
# baseline (speedup 1.0000x reference)
"""Trainium2 Bass kernel for nn_LinearQuantizerModel.

MLP 1024->894->763->501 (leaky_relu 0.01) + argmax over classes + exact
forward-fill of stop tokens (==500) done on host.

Sharding: data-parallel over batch B=16 across 8 cores (2 batches/core =
4000 tokens), weights replicated on device via an on-device AllGather of
1/8th weight shards (the axon host->device link is the bottleneck, so we
ship each byte once).

Transfer-minimal layout: x ships token-major in fp16 (65.5 MB total) and
is transposed on device by the DMA XBAR; weights ship as fp16 shards
(0.5 MB/core). fp16 matmuls accumulate in fp32 PSUM; max logit error vs
the fp32 reference is ~3e-4, so device argmax is exact except near-ties.
Tokens whose top-2 logit gap < GAP_T are recomputed exactly on host.
"""

import numpy as np

import concourse.bass as bass
import concourse.mybir as mybir
import concourse.tile as tile
from concourse import bacc
from concourse.bass_utils import run_bass_kernel_spmd

B, T, DIM, H1, H2, OUT = 16, 2000, 1024, 894, 763, 501
OUTP = 512            # class dim padded; pad classes get -30000 bias
VOCAB = 500
MAX_ITERS = 10000
NCORES = 8
RT = 4000             # tokens per core (exact, no padding)
CH = 400              # tokens per chunk (multiple of 16 for DMA XBAR)
NCHUNK = 10
SUB = 100             # tokens per argmax subtile (4 per chunk)
NSUB = 40             # code columns = NCHUNK * 4
KC1, MT1 = 8, 7       # DIM/128, ceil(H1/128)
KC2, MT2 = 7, 6       # ceil(H1/128), ceil(H2/128)
KC3 = 6               # ceil(H2/128)

W1N = 128 * KC1 * H1          # 915456
W2N = 128 * KC2 * 768         # 688128 (H2 padded to 768 free)
W3N = 128 * KC3 * OUTP        # 393216
WTOT = W1N + W2N + W3N + OUTP  # 1997312, divisible by 8
WSH = WTOT // NCORES           # 249664 per-core weight shard
XN = RT * DIM                  # 4096000
BN = 128 * (MT1 + MT2) * 2     # 3328: fp32 biases shipped as fp16 pairs
ON = SUB * 2 * NSUB            # 8000: per-core result elements

SHARD_W = True        # AllGather weight shards on device
GAP_T = 1e-3          # host-recompute threshold on top-2 logit gap

F16 = mybir.dt.float16
F32 = mybir.dt.float32

_CACHE = {}


def _install_fast_pjrt():
    """Replace bass2jax.run_bass_via_pjrt with a jit-memoizing equivalent.

    The stock implementation rebuilds jax.jit(shard_map(...)) on every call,
    so each run pays ~1s of re-trace + XLA re-compile, and it concatenates
    per-core inputs on host then pushes them through a slow sharded
    device_put path (~25 MB/s vs ~60 MB/s for direct per-device puts).
    This version caches the jit per Bass module and transfers each core's
    shard directly to its device. Inputs are still shipped and the NEFF
    still executes fully on every call.
    """
    if _CACHE.get("patched"):
        return
    import jax
    from jax.sharding import Mesh, NamedSharding, PartitionSpec
    from jax.experimental.shard_map import shard_map
    from concourse import bass2jax

    try:
        jax.config.update("jax_compilation_cache_dir", "/tmp/jax_comp_cache")
        jax.config.update("jax_persistent_cache_min_entry_size_bytes", -1)
        jax.config.update("jax_persistent_cache_min_compile_time_secs", 0)
    except Exception:
        pass

    orig = bass2jax.run_bass_via_pjrt
    jit_cache = {}

    def fast(nc, in_maps, n_cores):
        if n_cores == 1 or nc.dbg_addr is not None:
            return orig(nc, in_maps, n_cores)
        key = id(nc)
        if key not in jit_cache:
            bass2jax.install_neuronx_cc_hook()
            partition_name = (nc.partition_id_tensor.name
                              if nc.partition_id_tensor else None)
            in_names, out_names, out_avals, zero_shapes = [], [], [], []
            for alloc in nc.m.functions[0].allocations:
                if not isinstance(alloc, mybir.MemoryLocationSet):
                    continue
                name = alloc.memorylocations[0].name
                if alloc.kind == "ExternalInput":
                    if name != partition_name:
                        in_names.append(name)
                elif alloc.kind == "ExternalOutput":
                    shape = tuple(alloc.tensor_shape)
                    dtype = mybir.dt.np(alloc.dtype)
                    out_names.append(name)
                    out_avals.append(jax.core.ShapedArray(shape, dtype))
                    zero_shapes.append((shape, dtype))
            n_params = len(in_names)
            n_outs = len(out_avals)
            all_names = in_names + out_names + (
                [partition_name] if partition_name else [])
            donate = tuple(range(n_params, n_params + n_outs))

            def _body(*args):
                operands = list(args)
                if partition_name is not None:
                    operands.append(bass2jax.partition_id_tensor())
                outs = bass2jax._bass_exec_p.bind(
                    *operands, out_avals=tuple(out_avals),
                    in_names=tuple(all_names), out_names=tuple(out_names),
                    lowering_input_output_aliases=(),
                    sim_require_finite=True, sim_require_nnan=True, nc=nc)
                return tuple(outs)

            devices = jax.devices()[:n_cores]
            mesh = Mesh(np.asarray(devices), ("core",))
            # outputs named *_repl hold identical (AllGathered) values on
            # every core: expose them replicated so only one shard is pulled
            repl = [name.endswith("_repl") for name in out_names]
            out_specs = tuple(
                PartitionSpec() if r else PartitionSpec("core") for r in repl)
            sharded = jax.jit(
                shard_map(_body, mesh=mesh,
                          in_specs=(PartitionSpec("core"),) * (n_params + n_outs),
                          out_specs=out_specs,
                          check_rep=False),
                donate_argnums=donate, keep_unused=True)

            import jax.numpy as jnp
            zsh = tuple(NamedSharding(mesh, PartitionSpec("core"))
                        for _ in zero_shapes)

            def _mk_zeros():
                return tuple(
                    jnp.zeros((n_cores * s[0], *s[1:]), dt)
                    for s, dt in zero_shapes)

            zmaker = jax.jit(_mk_zeros, out_shardings=zsh)
            jit_cache[key] = (sharded, zmaker, in_names, out_names,
                             out_avals, repl, devices, mesh)
        (sharded, zmaker, in_names, out_names, out_avals, repl, devices,
         mesh) = jit_cache[key]
        n_cores_ = len(devices)
        sh = NamedSharding(mesh, PartitionSpec("core"))

        # per-device direct puts (fast path on the axon tunnel)
        g_ins = []
        for name in in_names:
            shards = [jax.device_put(np.asarray(m[name]), d)
                      for m, d in zip(in_maps, devices)]
            shape0 = shards[0].shape
            g_ins.append(jax.make_array_from_single_device_arrays(
                (n_cores_ * shape0[0], *shape0[1:]), sh, shards))
        g_zeros = zmaker()   # donated output buffers built on device
        out_arrs = sharded(*g_ins, *g_zeros)
        res = []
        fetched = [np.asarray(o) for o in out_arrs]
        for c in range(n_cores_):
            m = {}
            for i, name in enumerate(out_names):
                if repl[i]:
                    m[name] = fetched[i]
                else:
                    m[name] = fetched[i].reshape(
                        n_cores_, *out_avals[i].shape)[c]
            res.append(m)
        return res

    bass2jax.run_bass_via_pjrt = fast
    _CACHE["patched"] = True


def build_kernel(shard_w=SHARD_W):
    nc = bacc.Bacc(target_bir_lowering=False, num_devices=NCORES)

    nblob = XN + (WSH if shard_w else WTOT) + BN
    blob = nc.dram_tensor("blob", [nblob], F16, kind="ExternalInput")
    # "_repl" suffix: every core writes the identical AllGathered result,
    # so the host-side runner fetches a single replicated shard.
    out_d = nc.dram_tensor("out_repl", [NCORES * SUB, 2 * NSUB],
                           mybir.dt.int32, kind="ExternalOutput")
    out_b = nc.dram_tensor("outb", [ON], mybir.dt.int32)
    og = nc.dram_tensor("og", [NCORES * ON], mybir.dt.int32,
                        addr_space="Shared")

    if shard_w:
        wshb = nc.dram_tensor("wshb", [WSH], F16)
        wg = nc.dram_tensor("wg", [WTOT], F16, addr_space="Shared")

    LR = mybir.ActivationFunctionType.Lrelu

    with tile.TileContext(nc) as tc:
        with (
            tc.tile_pool(name="wpool", bufs=1) as wp,
            tc.tile_pool(name="xpool", bufs=3) as xp,
            tc.tile_pool(name="hpool", bufs=2) as hp,
            tc.tile_pool(name="spool", bufs=3) as sp,
            tc.tile_pool(name="cpool", bufs=1) as cp,
            tc.tile_pool(name="ps12", bufs=4, space="PSUM") as ps12,
            tc.tile_pool(name="ps3", bufs=3, space="PSUM") as ps3,
        ):
            if shard_w:
                nc.gpsimd.dma_start(out=wshb[:], in_=blob[XN:XN + WSH])
                nc.gpsimd.collective_compute(
                    "AllGather", mybir.AluOpType.bypass,
                    replica_groups=[list(range(NCORES))],
                    ins=[wshb[:].opt()], outs=[wg[:].opt()])
                wsrc, woff = wg, 0
            else:
                wsrc, woff = blob, XN

            # ---- weights / biases (loaded once) ----
            w1 = wp.tile([128, KC1, H1], F16)
            nc.sync.dma_start(
                out=w1,
                in_=wsrc[woff:woff + W1N].rearrange("(p r) -> p r", p=128))
            w2 = wp.tile([128, KC2, 768], F16)
            nc.sync.dma_start(
                out=w2,
                in_=wsrc[woff + W1N:woff + W1N + W2N].rearrange(
                    "(p r) -> p r", p=128))
            w3 = wp.tile([128, KC3, OUTP], F16)
            nc.sync.dma_start(
                out=w3,
                in_=wsrc[woff + W1N + W2N:woff + W1N + W2N + W3N].rearrange(
                    "(p r) -> p r", p=128))
            b3 = wp.tile([1, OUTP], F16)
            nc.sync.dma_start(
                out=b3,
                in_=wsrc[woff + W1N + W2N + W3N:woff + WTOT].rearrange(
                    "(o r) -> o r", o=1))
            boff = XN + (WSH if shard_w else WTOT)
            b12h = wp.tile([128, 2 * (MT1 + MT2)], F16)
            nc.sync.dma_start(
                out=b12h,
                in_=blob[boff:boff + BN].rearrange("(p r) -> p r", p=128))
            b12 = b12h.bitcast(F32)   # [128, MT1+MT2] fp32 view
            ones_f = wp.tile([1, SUB], F32)
            nc.vector.memset(ones_f, 1.0)
            ones = wp.tile([1, SUB], F16)
            nc.vector.tensor_copy(ones, ones_f)

            out_sb = cp.tile([SUB, 2 * NSUB], mybir.dt.int32)

            for c in range(NCHUNK):
                # transpose-load x chunk: xs[p, k, t] = x[c*CH+t, k*128+p]
                xs = xp.tile([128, KC1, CH], F16, tag="xs")
                nc.sync.dma_start_transpose(
                    out=xs,
                    in_=blob[c * CH * DIM:(c + 1) * CH * DIM].rearrange(
                        "(t f) -> t f", f=DIM))

                # ---- layer 1: h1t[m*128+p, t] ----
                h1t = hp.tile([128, KC2, CH], F16, tag="h1t")
                for mt in range(MT1):
                    m0 = mt * 128
                    mw = min(128, H1 - m0)
                    pt = ps12.tile([128, CH], F32, tag="pmm")
                    for kc in range(KC1):
                        nc.tensor.matmul(
                            pt[:mw, :], w1[:, kc, m0:m0 + mw], xs[:, kc, :],
                            start=(kc == 0), stop=(kc == KC1 - 1))
                    nc.scalar.activation(
                        h1t[:mw, mt, :], pt[:mw, :], LR,
                        bias=b12[:mw, mt:mt + 1], scale=1.0, alpha=0.01)

                # ---- layer 2 ----
                h2t = hp.tile([128, KC3, CH], F16, tag="h2t")
                for mt in range(MT2):
                    m0 = mt * 128
                    mw = min(128, H2 - m0)
                    pt = ps12.tile([128, CH], F32, tag="pmm")
                    for kc in range(KC2):
                        kw = min(128, H1 - kc * 128)
                        nc.tensor.matmul(
                            pt[:mw, :], w2[:kw, kc, m0:m0 + mw],
                            h1t[:kw, kc, :],
                            start=(kc == 0), stop=(kc == KC2 - 1))
                    nc.scalar.activation(
                        h2t[:mw, mt, :], pt[:mw, :], LR,
                        bias=b12[:mw, MT1 + mt:MT1 + mt + 1],
                        scale=1.0, alpha=0.01)

                # ---- layer 3 + argmax: per 100-token subtile ----
                for s in range(4):
                    t0 = s * SUB
                    pl = ps3.tile([128, OUTP], F32, tag="plog")
                    nc.tensor.matmul(pl[:SUB, :], ones, b3,
                                     start=True, stop=False)
                    for kc in range(KC3):
                        kw = min(128, H2 - kc * 128)
                        nc.tensor.matmul(
                            pl[:SUB, :], h2t[:kw, kc, t0:t0 + SUB],
                            w3[:kw, kc, :],
                            start=False, stop=(kc == KC3 - 1))
                    logit = sp.tile([128, OUTP], F32, tag="logit")
                    nc.scalar.copy(logit[:SUB, :], pl[:SUB, :])
                    mx8 = sp.tile([128, 8], F32, tag="mx8")
                    ix8 = sp.tile([128, 8], mybir.dt.uint32, tag="ix8")
                    nc.vector.max(mx8[:SUB, :], logit[:SUB, :])
                    nc.vector.max_index(ix8[:SUB, :], mx8[:SUB, :],
                                        logit[:SUB, :])
                    col = c * 4 + s
                    nc.vector.tensor_copy(
                        out_sb.bitcast(mybir.dt.uint32)[:, col:col + 1],
                        ix8[:SUB, 0:1])
                    nc.vector.tensor_sub(
                        out_sb.bitcast(F32)[:, NSUB + col:NSUB + col + 1],
                        mx8[:SUB, 0:1], mx8[:SUB, 1:2])

            # gather every core's result so each core holds the full output
            nc.sync.dma_start(out=out_b[:], in_=out_sb)
            nc.gpsimd.collective_compute(
                "AllGather", mybir.AluOpType.bypass,
                replica_groups=[list(range(NCORES))],
                ins=[out_b[:].opt()], outs=[og[:].opt()])
            nc.sync.dma_start(
                out=out_d[:],
                in_=og[:].rearrange("(a b) -> a b", b=2 * NSUB))

    nc.finalize()
    return nc


def _pack_weights(W1, b1, W2, b2, W3, b3):
    """Pack weights fp16 in the device block layout, flat, plus fp32 biases."""
    W1p = np.ascontiguousarray(
        W1.astype(np.float16).reshape(KC1, 128, H1).transpose(1, 0, 2))
    W2z = np.zeros((KC2 * 128, H2), np.float16)
    W2z[:H1] = W2.astype(np.float16)
    W2p = np.zeros((128, KC2, 768), np.float16)
    W2p[:, :, :H2] = W2z.reshape(KC2, 128, H2).transpose(1, 0, 2)
    W3z = np.zeros((KC3 * 128, OUTP), np.float16)
    W3z[:H2, :OUT] = W3.astype(np.float16)
    W3p = np.ascontiguousarray(
        W3z.reshape(KC3, 128, OUTP).transpose(1, 0, 2))
    b3p = np.full((OUTP,), -30000.0, np.float16)
    b3p[:OUT] = b3.astype(np.float16)
    flatW = np.concatenate(
        [W1p.ravel(), W2p.ravel(), W3p.ravel(), b3p])

    bias12 = np.zeros((128, MT1 + MT2), np.float32)
    b1z = np.zeros((MT1 * 128,), np.float32)
    b1z[:H1] = b1
    bias12[:, :MT1] = b1z.reshape(MT1, 128).T
    b2z = np.zeros((MT2 * 128,), np.float32)
    b2z[:H2] = b2
    bias12[:, MT1:] = b2z.reshape(MT2, 128).T
    return flatW, bias12


def _forward_fill_exact(code_flat: np.ndarray) -> np.ndarray:
    """Exact equivalent of the reference jax while-loop fill."""
    n = code_flat.shape[0]
    mask = code_flat == VOCAB
    if not mask.any():
        return code_flat
    if mask.all():
        return code_flat
    idx = np.where(~mask, np.arange(n), -1)
    fill = np.maximum.accumulate(idx)
    # wrap-around: positions before first non-stop take the last non-stop
    last = np.max(idx)
    dist = np.arange(n) - fill
    wrapped = fill < 0
    fill = np.where(wrapped, last, fill)
    dist = np.where(wrapped, np.arange(n) + (n - last), dist)
    out = code_flat[fill]
    # faithful MAX_ITERS cap: stops further than MAX_ITERS remain
    out = np.where(mask & (dist > MAX_ITERS), VOCAB, out)
    out = np.where(mask, out, code_flat)
    return out.astype(np.int32)


def kernel(x, W1, b1, W2, b2, W3, b3):
    x = np.asarray(x, dtype=np.float32)
    W1 = np.asarray(W1, dtype=np.float32)
    W2 = np.asarray(W2, dtype=np.float32)
    W3 = np.asarray(W3, dtype=np.float32)
    b1 = np.asarray(b1, dtype=np.float32)
    b2 = np.asarray(b2, dtype=np.float32)
    b3 = np.asarray(b3, dtype=np.float32)

    _install_fast_pjrt()
    if "nc" not in _CACHE:
        _CACHE["nc"] = build_kernel()
    nc = _CACHE["nc"]

    flatW, bias12 = _pack_weights(W1, b1, W2, b2, W3, b3)

    bias_h = bias12.reshape(-1).view(np.float16)     # fp32 -> fp16 pairs
    nw = WSH if SHARD_W else WTOT
    nblob = XN + nw + BN
    blob = np.empty((NCORES, nblob), np.float16)
    xr = x.reshape(NCORES, RT, DIM)
    for i in range(NCORES):
        blob[i, :XN].reshape(RT, DIM)[:] = xr[i]   # fp32 -> fp16 in place
        if SHARD_W:
            blob[i, XN:XN + nw] = flatW[i * WSH:(i + 1) * WSH]
        else:
            blob[i, XN:XN + nw] = flatW
        blob[i, XN + nw:] = bias_h

    in_maps = [{"blob": blob[i]} for i in range(NCORES)]
    _CACHE["in_maps"] = in_maps

    res = None
    for attempt in range(3):
        try:
            res = run_bass_kernel_spmd(nc, in_maps,
                                       core_ids=list(range(NCORES)))
            break
        except Exception:
            # transient NRT exec-unit wedge: cool down, then retry
            if attempt == 2:
                raise
            import time as _time
            _time.sleep(10)

    allout = res.results[0]["out_repl"]             # [NCORES*SUB, 2*NSUB]
    parts, gparts = [], []
    for i in range(NCORES):
        o = allout[i * SUB:(i + 1) * SUB]           # [SUB, 2*NSUB] int32
        parts.append(o[:, :NSUB].T.reshape(-1))     # token t = col*SUB + p
        gparts.append(np.ascontiguousarray(
            o[:, NSUB:]).view(np.float32).T.reshape(-1))
    code = np.concatenate(parts).astype(np.int32)   # [32000]
    gap = np.concatenate(gparts).astype(np.float32)

    # fp16 matmul can flip near-ties; recompute uncertain tokens exactly
    unc = np.flatnonzero(gap < GAP_T)
    if unc.size:
        xf = x.reshape(-1, DIM)[unc].astype(np.float32)
        h = xf @ W1 + b1
        h = np.where(h >= 0, h, np.float32(0.01) * h).astype(np.float32)
        h = h @ W2 + b2
        h = np.where(h >= 0, h, np.float32(0.01) * h).astype(np.float32)
        lg = h @ W3 + b3
        code[unc] = np.argmax(lg, axis=-1).astype(np.int32)

    code = _forward_fill_exact(code)
    return code.reshape(B, T)
